# revision 16
# baseline (speedup 1.0000x reference)
"""Trainium2 Bass kernel for nn_Candidate_Scorer.

Reference computation:
    b = G_p @ wb            # [N,1]
    e = G_p @ we            # [N,1]
    num = exp(b + e.T)      # [N,N]
    den = sum(num)
    P = triu(num / den)
    top_k(P.reshape(-1), k) -> ((row, col) indices, values)

Key structure exploited:
  * num = exp(b) * exp(e).T is rank-1, so den = sum(exp(b)) * sum(exp(e)).
    No N x N reduction is needed.
  * exp is monotone, so the top-k of exp(b_i + e_j) over {j >= i} is the
    top-k of b_i + e_j over the same set -- selected from the two
    N-vectors with an exact thresholding argument (see _select_topk).

Device work (SPMD over 8 cores, rows sharded): b = G@wb, e = G@we as
batched multiply + reduce on the Vector engine (rows on partitions -
avoids the PE fp32 4-cycles/row penalty, keeps full f32 accuracy), exp
on the Scalar engine, softmax-denominator partials via a final Vector
reduce.  Raw engine programs (bacc) with manual semaphores; input is
pipelined over three DMA channels (sync HWDGE, scalar HWDGE, gpsimd
SWDGE) so compute starts after the first 205 KB lands.
Host work (gather/merge): concatenate shards, exact top-k candidate
selection from the N-vectors, final value/index assembly.
"""

import numpy as np

N = 8192
D = 200
N_CORES = 8
ROWS = N // N_CORES    # 1024 rows per core
BLK = ROWS // 128      # 8 row-blocks of 128 partitions

# SBUF tile layout [128, 2000]:
#   cols 0:200      wb broadcast to all partitions
#   cols 200:400    G block 0 (row blk*128+p at partition p)
#   cols 400:600    we broadcast
#   cols 600:2000   G blocks 1..7
# DMA chunks: sync ring:  cols 0:400 then 400:600
#             scalar ring: cols 600:1200  (blocks 1-3)
#             gpsimd ring: cols 1200:2000 (blocks 4-7)
R1C, R2C, R3C = 600, 600, 800

_COMPILED = {}


def _build_program():
    """Per-core SPMD program (bacc, manual sync).

    Inputs (per core):  "gw1" [128,600], "gw2" [128,600], "gw3" [128,800]
    Outputs (per core): "out"  [128, 16] f32: cols v*8+blk = b/e values
                        "out2" [128, 2]  f32: per-partition exp sums
    """
    import concourse.bass as bass
    import concourse.bacc as bacc
    import concourse.mybir as mybir

    dt = mybir.dt.float32
    fexp = mybir.ActivationFunctionType.Exp
    nc = bacc.Bacc("TRN2", target_bir_lowering=False, debug=False,
                   num_devices=N_CORES)

    gw1_d = nc.dram_tensor("gw1", [128, R1C], dt, kind="ExternalInput")
    gw2_d = nc.dram_tensor("gw2", [128, R2C], dt, kind="ExternalInput")
    gw3_d = nc.dram_tensor("gw3", [128, R3C], dt, kind="ExternalInput")
    out_d = nc.dram_tensor("out", [128, 16], dt, kind="ExternalOutput")
    out2_d = nc.dram_tensor("out2", [128, 2], dt, kind="ExternalOutput")

    with (
        nc.sbuf_tensor("gw_s", [128, 2000], dt) as gw_s,
        nc.sbuf_tensor("out_s", [128, 16], dt) as out_s,
        nc.sbuf_tensor("out2_s", [128, 2], dt) as out2_s,
        nc.sbuf_tensor("prod_s", [128, 2 * 4 * D], dt) as prod_s,
        nc.sbuf_tensor("ebe_s", [128, 2 * BLK], dt) as ebe_s,
        nc.semaphore("s_r1") as s_r1,
        nc.semaphore("s_r2") as s_r2,
        nc.semaphore("s_r3") as s_r3,
        nc.semaphore("s_dve") as s_dve,
        nc.semaphore("s_act") as s_act,
        nc.semaphore("s_done") as s_done,
        nc.semaphore("s_out") as s_out,
        nc.Block() as block,
    ):
        # [128, 2, 200] view selecting wb (col 0) and we (col 400)
        w_pair = gw_s[:, 0:600].rearrange("p (v d) -> p v d", v=3)[:, 0::2, :]
        # out columns viewed as [2, BLK]
        bev = out_s[:].rearrange("p (v z) -> p v z", v=2)

        def blk_cols(z0, nb):
            # G block columns: block 0 at 200, blocks 1.. at 600
            c0 = 200 + z0 * D if z0 == 0 else 400 + z0 * D
            return gw_s[:, c0:c0 + nb * D]

        @block.sync
        def _(sync):
            sync.dma_start(gw_s[:, 0:400], gw1_d[:, 0:400]).then_inc(s_r1, 16)
            sync.dma_start(gw_s[:, 400:600], gw1_d[:, 400:600]
                           ).then_inc(s_r1, 16)
            sync.wait_ge(s_dve, 1)
            sync.dma_start(out_d[:], out_s[:]).then_inc(s_out, 16)
            sync.wait_ge(s_done, 1)
            sync.dma_start(out2_d[:], out2_s[:]).then_inc(s_out, 16)
            sync.wait_ge(s_out, 32)

        @block.scalar
        def _(scalar):
            # blocks 1-3 then 4-7 on the ACT HWDGE ring, parallel w/ ring1
            scalar.dma_start(gw_s[:, 600:1200], gw2_d[:]).then_inc(s_r2, 16)
            scalar.dma_start(gw_s[:, 1200:2000], gw3_d[:]).then_inc(s_r3, 16)
            # warm the Exp table while the DMAs fly (result discarded)
            nc.scalar.activation(ebe_s[:, 0:1], out_s[:, 0:1], fexp)
            scalar.wait_ge(s_dve, 1)
            nc.scalar.activation(ebe_s[:], out_s[:], fexp).then_inc(s_act, 1)

        @block.vector
        def _(vector):
            # (v or None=both, z0, nb, sem, threshold)
            plan = [(None, 0, 1, s_r1, 32),
                    (None, 1, 3, s_r2, 16), (None, 4, 4, s_r3, 16)]
            for v, z0, nb, sem, thr in plan:
                vector.wait_ge(sem, thr)
                g = blk_cols(z0, nb).rearrange("p (z d) -> p z d", z=nb)
                if v is None:
                    g4 = (g.rearrange("p z (u d) -> p u z d", u=1)
                          .broadcast_to((128, 2, nb, D)))
                    w4 = (w_pair.rearrange("p v (z d) -> p v z d", z=1)
                          .broadcast_to((128, 2, nb, D)))
                    p4 = (prod_s[:, 0:2 * nb * D]
                          .rearrange("p (v z d) -> p v z d", v=2, z=nb))
                    o4 = bev[:, :, z0:z0 + nb]
                else:
                    g4 = g
                    w4 = (w_pair[:, v:v + 1, :]
                          .broadcast_to((128, nb, D)))
                    p4 = (prod_s[:, 0:nb * D]
                          .rearrange("p (z d) -> p z d", z=nb))
                    o4 = bev[:, v, z0:z0 + nb]
                nc.vector.tensor_tensor(p4, g4, w4, op=mybir.AluOpType.mult)
                ins = nc.vector.reduce_sum(o4, p4, axis=mybir.AxisListType.X)
            ins.then_inc(s_dve, 1)
            # softmax-denominator partials: per-partition sums of exp
            vector.wait_ge(s_act, 1)
            e3 = ebe_s[:].rearrange("p (v z) -> p v z", v=2)
            nc.vector.reduce_sum(out2_s[:], e3, axis=mybir.AxisListType.X
                                 ).then_inc(s_done, 1)

    nc.compile()
    return nc


def _get_program():
    if "nc" not in _COMPILED:
        _COMPILED["nc"] = _build_program()
    return _COMPILED["nc"]


def _pack_inputs(G_p, wb, we):
    wb = wb.reshape(-1).astype(np.float32)
    we = we.reshape(-1).astype(np.float32)
    in_maps = []
    for c in range(N_CORES):
        shard = G_p[c * ROWS:(c + 1) * ROWS, :].astype(np.float32)
        blocks = shard.reshape(BLK, 128, D).transpose(1, 0, 2)  # [128,8,200]
        gw1 = np.empty((128, R1C), dtype=np.float32)
        gw1[:, 0:D] = wb[None, :]
        gw1[:, D:2 * D] = blocks[:, 0, :]
        gw1[:, 2 * D:3 * D] = we[None, :]
        gw2 = np.ascontiguousarray(blocks[:, 1:4, :].reshape(128, 3 * D))
        gw3 = np.ascontiguousarray(blocks[:, 4:8, :].reshape(128, 4 * D))
        in_maps.append({"gw1": gw1, "gw2": gw2, "gw3": gw3})
    return in_maps


def _run_device(G_p, wb, we, trace=False):
    from concourse.bass_utils import run_bass_kernel_spmd

    nc = _get_program()
    in_maps = _pack_inputs(G_p, wb, we)
    res = run_bass_kernel_spmd(nc, in_maps, core_ids=list(range(N_CORES)),
                               trace=trace)
    return res


def _select_topk(b, e, den, k):
    """Exact top-k of exp(b_i + e_j)/den over {(i, j): j >= i}.

    Threshold argument: rowbest[i] = b[i] + max(e[i:]) is each row's best
    pair value. The k-th largest rowbest T is a lower bound on the k-th
    largest pair value (k distinct rows each contain a pair >= T), so
    every true top-k pair has value >= T. We enumerate all valid pairs
    with b_i + e_j >= T (minus a small safety margin) and rank them
    exactly as jax.lax.top_k does: by f32 value descending, ties broken
    by lower flat index.
    """
    bf = b.astype(np.float32)
    ef = e.astype(np.float32)
    n = bf.shape[0]

    suff = np.maximum.accumulate(ef[::-1])[::-1]   # suffix max of e
    rowbest = bf + suff
    kth = np.partition(rowbest, n - k)[n - k] - np.float32(1e-4)

    order_e = np.lexsort((np.arange(n), -ef))
    e_sorted = ef[order_e]

    rows = np.where(rowbest >= kth)[0]
    cand_i, cand_j = [], []
    for i in rows:
        t = kth - bf[i]
        cnt = int(np.searchsorted(-e_sorted, -t, side="right"))
        if cnt == 0:
            continue
        js = order_e[:cnt]
        js = js[js >= i]
        if js.size:
            cand_i.append(np.full(js.size, i, dtype=np.int64))
            cand_j.append(js)
    ci = np.concatenate(cand_i)
    cj = np.concatenate(cand_j)

    # values exactly as the reference computes them: f32 add, f32 exp,
    # f32 divide
    s = (bf[ci] + ef[cj]).astype(np.float32)
    v = np.exp(s).astype(np.float32) / np.float32(den)
    flat = ci * n + cj
    order = np.lexsort((flat, -v))[:k]
    top_i = ci[order]
    top_j = cj[order]
    idx = np.stack([top_i, top_j], axis=1).astype(np.int32)
    return idx, v[order].astype(np.float32)


def kernel(G_p, wb, we, k):
    G_p = np.asarray(G_p, dtype=np.float32)
    wb = np.asarray(wb, dtype=np.float32).reshape(D, 1)
    we = np.asarray(we, dtype=np.float32).reshape(D, 1)
    k = int(k)

    res = _run_device(G_p, wb, we)
    outs = res.results

    # out[:, v*8+blk] at partition p = b/e[blk*128 + p]
    b = np.concatenate(
        [outs[c]["out"][:, 0:BLK].T.reshape(-1) for c in range(N_CORES)])
    e = np.concatenate(
        [outs[c]["out"][:, BLK:2 * BLK].T.reshape(-1) for c in range(N_CORES)])
    S_b = np.float32(sum(outs[c]["out2"][:, 0].sum(dtype=np.float64)
                         for c in range(N_CORES)))
    S_e = np.float32(sum(outs[c]["out2"][:, 1].sum(dtype=np.float64)
                         for c in range(N_CORES)))
    den = np.float32(S_b * S_e)

    idx, vals = _select_topk(b, e, den, k)
    return idx, vals
